# revision 1
# baseline (speedup 1.0000x reference)
"""Neighbor aggregation (GNN message passing) on 8 Trainium2 cores — v14.

vs v5: asymmetric window split per dst block — 8 groups from the low src
window [0, 32768) + 9 groups from the high window [17232, 50048), with
flexible middle-range slots spilled to the high side by capacity.  17 groups
per block instead of 18: 5.7% fewer gather slots/calls/W bytes/matmuls.
Dedup is side-agnostic by (block, src); W columns are multi-hot.
"""

import sys

sys.path.insert(0, "/opt/trn_rl_repo")

import numpy as np

import concourse.bacc as bacc
import concourse.tile as tile
from concourse import mybir
from concourse.bass_utils import run_bass_kernel_spmd

B = 4
N_NODES = 50000
HS = 16
C = HS * HS
P = 128
NBLK = 196
HALF0 = NBLK * P
GLO_LO = 8                # groups per block from the low window (cap 1024)
GLO_HI = 9                # groups per block from the high window (cap 1152)
GPB = GLO_LO + GLO_HI     # 17
NGRP = NBLK * GPB         # 3332
HI_BASE = 17232
N_QUEUES = 4
SB = 8

_prog_cache: dict = {}
_last_in_maps: list | None = None


def _group_order():
    """canonical group id (block*GPB + side_off + k) -> stream position."""
    order = []
    for sb0 in range(0, NBLK, SB):
        nb = min(SB, NBLK - sb0)
        for s, glo in ((0, GLO_LO), (1, GLO_HI)):
            off = 0 if s == 0 else GLO_LO
            for jj in range(nb):
                for k in range(glo):
                    order.append((sb0 + jj) * GPB + off + k)
    pos = np.empty(NGRP, np.int64)
    pos[np.asarray(order)] = np.arange(NGRP)
    return pos


POS_OF_GROUP = _group_order()
_GB = np.empty(NGRP, np.int64)
_GB[POS_OF_GROUP] = np.repeat(np.arange(NBLK), GPB)
GROUP_BLOCK = _GB


def _build_program(maxnlo):
    nc = bacc.Bacc("TRN2", target_bir_lowering=False, debug=False,
                   num_swdge_queues=N_QUEUES)
    h_d = nc.dram_tensor("h", (50048, C), mybir.dt.float16, kind="ExternalInput")
    idx_d = nc.dram_tensor("idx", (P, NGRP * 8), mybir.dt.int16, kind="ExternalInput")
    w_d = nc.dram_tensor("w", (P, NGRP * P), mybir.dt.float16, kind="ExternalInput")
    out_d = nc.dram_tensor("out", (NBLK * P, C), mybir.dt.float32, kind="ExternalOutput")

    h_ap = h_d.ap()
    win_aps = (h_ap[0:32768, :], h_ap[HI_BASE:HI_BASE + 32768, :])
    qctr = [0]

    with tile.TileContext(nc) as tc:
        with tc.tile_pool(name="const", bufs=1) as cpool, \
             tc.tile_pool(name="gat", bufs=20) as gpool, \
             tc.tile_pool(name="gat4", bufs=4) as g4pool, \
             tc.tile_pool(name="wt", bufs=8) as wpool, \
             tc.tile_pool(name="otile", bufs=4) as opool, \
             tc.tile_pool(name="psum", bufs=8, space="PSUM") as ppool:
            idx_t = cpool.tile([P, NGRP * 8], mybir.dt.int16)
            nc.sync.dma_start(out=idx_t[:], in_=idx_d.ap())

            # zero-init rotating gather buffers: slots skipped by the
            # trailing-negative-index trim keep stale contents; W columns are
            # zero there, but stale NaN*0 would poison PSUM
            for _ in range(20):
                t = gpool.tile([P, 8, C], mybir.dt.float16, tag="g8")
                nc.vector.memset(t[:], 0.0)
            for _ in range(4):
                t = g4pool.tile([P, 4, C], mybir.dt.float16, tag="g4")
                nc.vector.memset(t[:], 0.0)

            pos0 = 0
            for sb0 in range(0, NBLK, SB):
                nb = min(SB, NBLK - sb0)
                gtiles = [[], []]
                wtiles = []
                # interleave lo/hi gather calls so each block's full group set
                # (8 lo + 9 hi) lands early, letting matmuls retire tiles sooner
                calls = []
                for s, glo in ((0, GLO_LO), (1, GLO_HI)):
                    run = nb * glo
                    base = pos0 if s == 0 else pos0 + nb * GLO_LO
                    calls.append([(s, base + c0, min(8, run - c0))
                                  for c0 in range(0, run, 8)])
                    for c0 in range(0, run, 16):
                        nw = min(16, run - c0)
                        wt = wpool.tile([P, nw, P], mybir.dt.float16,
                                        tag=f"w{nw}")
                        nc.scalar.dma_start(
                            out=wt[:],
                            in_=w_d.ap()[:, (base + c0) * P:(base + c0 + nw) * P])
                        wtiles.append((wt, base + c0, nw))
                seq = []
                for i in range(max(len(calls[0]), len(calls[1]))):
                    for s in (0, 1):
                        if i < len(calls[s]):
                            seq.append(calls[s][i])
                for s, g0, ng in seq:
                    gp = gpool if ng == 8 else g4pool
                    t = gp.tile([P, ng, C], mybir.dt.float16, tag=f"g{ng}")
                    if s == 0 and ng == 8:
                        blk_of_call = GROUP_BLOCK[g0]
                        reg = int(maxnlo[blk_of_call])
                    else:
                        reg = ng * P
                    nc.gpsimd.dma_gather(
                        out_ap=t[:],
                        in_ap=win_aps[s],
                        idxs_ap=idx_t[:, g0 * 8:(g0 + ng) * 8],
                        num_idxs=ng * P,
                        num_idxs_reg=reg,
                        elem_size=C,
                        queue_num=qctr[0] % N_QUEUES,
                    )
                    qctr[0] += 1
                    gtiles[s].append((t, g0, ng))

                for jj in range(nb):
                    acc = ppool.tile([P, C], mybir.dt.float32, space="PSUM")
                    for kk in range(GPB):
                        if kk < GLO_LO:
                            s = 0
                            pos = pos0 + jj * GLO_LO + kk
                        else:
                            s = 1
                            pos = (pos0 + nb * GLO_LO + jj * GLO_HI
                                   + (kk - GLO_LO))
                        t, t0, _ = next(p for p in gtiles[s]
                                        if p[1] <= pos < p[1] + p[2])
                        wt, w0, _ = next(p for p in wtiles
                                         if p[1] <= pos < p[1] + p[2])
                        nc.tensor.matmul(
                            out=acc[:], lhsT=wt[:, pos - w0, :],
                            rhs=t[:, pos - t0, :],
                            start=(kk == 0), stop=(kk == GPB - 1))
                    j = sb0 + jj
                    ot = opool.tile([P, C], mybir.dt.float32, tag="out")
                    nc.vector.tensor_copy(out=ot[:], in_=acc[:])
                    nc.sync.dma_start(out=out_d.ap()[j * P:(j + 1) * P, :],
                                      in_=ot[:])
                pos0 += nb * GPB

    nc.compile()
    return nc


def kernel(H, edge_index, edge_weight, node_idx):
    H = np.asarray(H, dtype=np.float32)
    edge_index = np.asarray(edge_index)
    edge_weight = np.ascontiguousarray(np.asarray(edge_weight), dtype=np.float32)
    node_idx = np.asarray(node_idx)

    inv = np.argsort(node_idx).astype(np.int64)

    preps = []
    for core in range(2 * B):
        b, half = divmod(core, 2)
        dst = inv[edge_index[b, :, 0]]
        src = inv[edge_index[b, :, 1]]
        w = edge_weight[b]
        m = (dst >= half * HALF0) & (dst < (half + 1) * HALF0)
        d = (dst[m] - half * HALF0).astype(np.int64)
        s = src[m]
        wv = w[m]

        blk = d >> 7
        pair = blk * 50000 + s              # dedup (block, src), side-agnostic
        uniq, inv_e = np.unique(pair, return_inverse=True)
        ublk = uniq // 50000
        usrc = uniq % 50000

        n_bl = np.bincount(ublk, minlength=NBLK)
        lo_strict = usrc < HI_BASE
        nlo_strict = np.bincount(ublk[lo_strict], minlength=NBLK)
        if n_bl.max() > GLO_LO * P + GLO_HI * P or nlo_strict.max() > GLO_LO * P:
            raise RuntimeError(f"block overflow: {n_bl.max()}, {nlo_strict.max()}")

        # middle slots [HI_BASE, 32768): fill the HIGH side to capacity so
        # padding lands on the lo side, whose calls are block-aligned (the
        # trailing-negative trim can skip lo pads).  First (n-1152-nlo_strict)
        # middle slots go lo (keeps side monotone within a block), rest hi.
        mid = (~lo_strict) & (usrc < 32768)
        mstarts = np.zeros(NBLK, np.int64)
        mcounts = np.bincount(ublk[mid], minlength=NBLK)
        mstarts[1:] = np.cumsum(mcounts)[:-1]
        mrank = np.zeros(len(uniq), np.int64)
        mrank[mid] = np.arange(int(mid.sum())) - mstarts[ublk[mid]]
        mid_to_lo = np.maximum(0, n_bl - GLO_HI * P - nlo_strict)
        side = np.ones(len(uniq), np.int64)
        side[lo_strict] = 0
        side[mid] = (mrank[mid] >= mid_to_lo[ublk[mid]]).astype(np.int64)

        # per-(block, side) counts & capacity check; uniq already sorted by
        # (block, src) and side is monotone within a block -> bucket sorted
        bucket = ublk * 2 + side
        counts = np.bincount(bucket, minlength=NBLK * 2)
        if counts[0::2].max() > GLO_LO * P or counts[1::2].max() > GLO_HI * P:
            raise RuntimeError("side overflow")
        starts = np.zeros(NBLK * 2, np.int64)
        starts[1:] = np.cumsum(counts)[:-1]
        urank = np.arange(len(uniq)) - starts[bucket]
        canon = ublk * GPB + side * GLO_LO + (urank // P)
        upos = POS_OF_GROUP[canon]
        uerow = urank % P

        preps.append((counts, uniq, inv_e, ublk, usrc, side, upos, uerow,
                      d, wv, b))
    maxnlo = np.zeros(NBLK, np.int64)
    for pr in preps:
        maxnlo = np.maximum(maxnlo, pr[0][0::2])

    in_maps = []
    for (counts, uniq, inv_e, ublk, usrc, side, upos, uerow,
         d, wv, b) in preps:
        sl = np.zeros(NGRP * P, np.int16)
        # beyond the cross-core max lo count, pads are trailing within the
        # block-aligned lo call AND consistent across cores: mark -1 so the
        # ucode trims them (num_idxs_reg matches maxnlo per call)
        nlo_b = maxnlo
        bl_all = np.repeat(np.arange(NBLK), GPB)
        off_all = np.tile(np.arange(GPB), NBLK)
        g_lo = off_all < GLO_LO
        canon_lo = (bl_all * GPB + off_all)[g_lo]
        rank0 = (off_all[g_lo] * P)
        spos = POS_OF_GROUP[canon_lo]
        for r in range(P):
            pass  # vectorized below
        rank_mat = rank0[:, None] + np.arange(P)[None, :]
        padm = rank_mat >= nlo_b[bl_all[g_lo], None]
        flat = (spos[:, None] * P + np.arange(P)[None, :])[padm]
        sl[flat] = -1
        sl[upos * P + uerow] = (usrc - side * HI_BASE).astype(np.int16)
        idx16 = sl.reshape(NGRP, 8, 16).transpose(2, 0, 1).reshape(16, NGRP * 8)
        idx128 = np.ascontiguousarray(np.tile(idx16, (8, 1)))

        wbig32 = np.zeros((P, NGRP * P), np.float32)
        np.add.at(wbig32, (uerow[inv_e], upos[inv_e] * P + (d & 127)), wv)
        wbig = wbig32.astype(np.float16)

        h16 = np.zeros((50048, C), np.float16)
        h16[:N_NODES] = H[b].reshape(N_NODES, C).astype(np.float16)

        in_maps.append({"h": h16, "idx": idx128, "w": wbig})

    global _last_in_maps
    _last_in_maps = in_maps
    key = ("v14", tuple(maxnlo.tolist()))
    nc = _prog_cache.get(key)
    if nc is None:
        nc = _build_program(maxnlo)
        _prog_cache[key] = nc

    res = run_bass_kernel_spmd(nc, in_maps, list(range(2 * B)))

    out = np.empty((B, N_NODES, HS, HS), np.float32)
    for b in range(B):
        r0 = res.results[2 * b]["out"]
        r1 = res.results[2 * b + 1]["out"]
        out[b, :HALF0] = r0.reshape(-1, HS, HS)
        out[b, HALF0:] = r1[:N_NODES - HALF0].reshape(-1, HS, HS)
    return out

